# revision 1
# baseline (speedup 1.0000x reference)
"""3-layer GCN + global mean pool, distributed over 8 Trainium2 NeuronCores.

Strategy (see spec sharding hint):
- Nodes are partitioned into 8 contiguous shards (12500 real nodes each,
  padded to 12544 = 98 tiles of 128).
- Per layer: every core computes u = dinv * (h @ W) for its shard on the
  TensorEngine, the u-table is AllGathered to every core's HBM, and each
  core gathers u[src] rows (dma_gather, int16 indices -> 4 source blocks)
  for the edges whose dst lives in its shard.  The segment-sum over
  incoming edges is done as a sequence of one-hot ("staircase") matmuls
  accumulating in PSUM: stair[e, v] = (dstslot[e] == v), generated on the
  fly on the DVE by comparing an iota row against per-edge dst slots.
  Padding edges carry dstslot = -1 and thus contribute zero.
- Mean pool: per node tile, a [128, 1024] one-hot over global graph ids
  feeds 8 accumulating matmuls into a [128, 1024] PSUM tile; partial sums
  are AllReduced across cores, divided by counts, and pushed through the
  final linear layer (replicated on every core).

The edge structure (chunk counts per tile/block) is made uniform across
cores so a single SPMD program serves all 8 cores; per-core data
(indices, dst slots, degrees, batch slots) is shipped as input tensors.
"""

import math

import numpy as np

import concourse.bacc as bacc
import concourse.bass as bass
import concourse.mybir as mybir
import concourse.tile as tile
from concourse.bass_utils import run_bass_kernel_spmd

P = 128
NCORES = 8
F32 = mybir.dt.float32
I16 = mybir.dt.int16


def _ceil_div(a, b):
    return (a + b - 1) // b


def _preprocess(x, edge_index, batch, n_graphs, group_tiles=4, maxidx=32767):
    """Build per-core input tensors + uniform static structure (meta)."""
    N = x.shape[0]
    DIN = x.shape[1]
    SR = N // NCORES              # real nodes per shard
    assert SR * NCORES == N
    TPC = _ceil_div(SR, P)        # tiles per core
    S = TPC * P                   # padded shard rows
    BLKS = maxidx // S            # shards per index block (int16 range)
    NBLK = _ceil_div(NCORES, BLKS)
    BLKR = BLKS * S               # rows per index block
    GT = group_tiles
    NG = _ceil_div(TPC, GT)
    GP = _ceil_div(n_graphs, P) * P   # padded graph count (1024)
    NGT = GP // P

    # self-loops are NOT materialized as edges: the tile's own u is added
    # algebraically on-device (saves one gather index per node per layer).
    src = np.asarray(edge_index[0], dtype=np.int64)
    dst = np.asarray(edge_index[1], dtype=np.int64)
    deg = (np.bincount(dst, minlength=N) + 1).astype(np.float32)  # + self

    src_row = (src // SR) * S + (src % SR)     # row in the gathered u table
    dst_core = dst // SR
    dst_slot = dst % SR

    batch = np.asarray(batch, dtype=np.int64)

    # ---- per (core, tile, block) edge counts, then uniform chunk counts
    tile_of = dst_slot // P
    blk_of = src_row // BLKR
    counts = np.zeros((NCORES, TPC, NBLK), dtype=np.int64)
    np.add.at(counts, (dst_core, tile_of, blk_of), 1)
    ch = _ceil_div(counts.max(axis=0), 1)  # per-core max below
    ch = np.ceil(counts.max(axis=0) / P).astype(np.int64)      # [TPC, NBLK]
    ch = np.maximum(ch, (counts.max(axis=0) > 0))              # safety
    ch[:, 0] = np.maximum(ch[:, 0], 1)   # every tile gets >=1 chunk
    TOTCH = int(ch.sum())

    # ---- group/chunk layout (uniform across cores)
    # msg-buffer chunks are laid out block-major (to match the per-block
    # gather calls); dslot columns are laid out TILE-major so each tile's
    # staircase op reads a contiguous dslot slice.  tmbase[t] is the
    # tile-major dslot column base for tile t.
    groups = []
    chunk_cursor = 0   # tile-major dslot column cursor
    idx_cursor16 = 0
    tmbase = {}
    for t in range(TPC):
        tmbase[t] = chunk_cursor
        chunk_cursor += int(ch[t, :].sum())
    assert chunk_cursor == TOTCH
    for g in range(NG):
        tiles = list(range(g * GT, min((g + 1) * GT, TPC)))
        blocks = []
        tile_chunks = {t: [] for t in tiles}   # local msg-chunk ids, in
        local = 0                              # tile-major emission order
        for b in range(NBLK):
            nb = int(sum(ch[t, b] for t in tiles))
            blocks.append((idx_cursor16, nb * P, local))
            for t in tiles:
                for _ in range(int(ch[t, b])):
                    tile_chunks[t].append(local)
                    local += 1
            idx_cursor16 += nb * P // 16
        groups.append({
            "tiles": [(t, tile_chunks[t], tmbase[t]) for t in tiles],
            "blocks": blocks,
            "n_chunks": local,
        })
    TOT16 = idx_cursor16
    CHMAX = max(g["n_chunks"] for g in groups)

    # ---- per-core tensors
    in_maps = []
    # order edges once per core by (tile, block); stable order inside
    for c in range(NCORES):
        m = dst_core == c
        e_row = src_row[m]
        e_tile = tile_of[m]
        e_blk = blk_of[m]
        e_slot = (dst_slot[m] % P).astype(np.float32)
        order = np.lexsort((e_blk, e_tile))
        e_row, e_tile, e_blk, e_slot = (
            e_row[order], e_tile[order], e_blk[order], e_slot[order])
        # bucket boundaries
        key = e_tile * NBLK + e_blk
        bnd = np.searchsorted(key, np.arange(TPC * NBLK + 1))
        idx_vals = np.zeros(TOT16 * 16, dtype=np.int16)
        dslot_vals = np.full(TOTCH * P, -1.0, dtype=np.float32)
        pos = 0   # position in idx_vals stream (gather order: block-major)
        dpos = {t: 0 for t in range(TPC)}   # per-tile dslot chunks written
        for gi, g in enumerate(groups):
            for b_i, (off16, n_idx, local0) in enumerate(g["blocks"]):
                assert pos == off16 * 16
                for (t, _tch, tmb) in g["tiles"]:
                    k = t * NBLK + b_i
                    lo, hi = bnd[k], bnd[k + 1]
                    n_pad = int(ch[t, b_i]) * P
                    seg_idx = np.zeros(n_pad, dtype=np.int16)
                    seg_idx[: hi - lo] = (e_row[lo:hi] % BLKR).astype(np.int16)
                    seg_slot = np.full(n_pad, -1.0, dtype=np.float32)
                    seg_slot[: hi - lo] = e_slot[lo:hi]
                    idx_vals[pos: pos + n_pad] = seg_idx
                    # dslot goes to the tile-major column range
                    d0 = (tmb + dpos[t]) * P
                    dslot_vals[d0: d0 + n_pad] = seg_slot
                    dpos[t] += int(ch[t, b_i])
                    pos += n_pad
        assert pos == TOT16 * 16
        # wrap indices: idx i -> [i % 16, i // 16], replicated to 128 rows
        idx_w = idx_vals.reshape(-1, 16).T.copy()          # [16, TOT16]
        idx_w = np.tile(idx_w, (8, 1))                     # [128, TOT16]
        # dslot: [128, TOTCH] : chunk j partition p = edge j*128+p
        dslot_w = dslot_vals.reshape(TOTCH, P).T.copy()

        nodes = np.arange(c * SR, (c + 1) * SR)
        deg_flat = np.ones(S, dtype=np.float32)
        deg_flat[:SR] = deg[nodes]
        deg_w = deg_flat.reshape(TPC, P).T.copy()
        pool_flat = np.full(S, -1.0, dtype=np.float32)
        pool_flat[:SR] = batch[nodes].astype(np.float32)
        pool_w = pool_flat.reshape(TPC, P).T.copy()

        xT = np.zeros((DIN, S), dtype=np.float32)
        xT[:, :SR] = np.asarray(x[nodes], dtype=np.float32).T

        cnt = np.bincount(batch, minlength=n_graphs).astype(np.float32)
        cnt_flat = np.ones(GP, dtype=np.float32)
        cnt_flat[:n_graphs] = cnt
        cnt_w = cnt_flat.reshape(NGT, P).T.copy()

        iota = np.broadcast_to(
            np.arange(GP, dtype=np.float32)[None, :], (P, GP)).copy()

        in_maps.append({
            "xT": xT, "idx": idx_w, "dslot": dslot_w, "deg": deg_w,
            "pslot": pool_w, "cnt": cnt_w, "iota": iota,
        })

    meta = dict(N=N, DIN=DIN, SR=SR, S=S, TPC=TPC, NBLK=NBLK, BLKR=BLKR,
                GT=GT, NG=NG, GP=GP, NGT=NGT, TOTCH=TOTCH, TOT16=TOT16,
                CHMAX=CHMAX, groups=groups, n_graphs=n_graphs)
    return in_maps, meta


def _build(meta, weights, msg_bufs=3, stage=4, use_bf16=False):
    """Build the SPMD Bass program. weights: dict of numpy arrays (same on
    every core) -> shipped as inputs."""
    SR, S, TPC = meta["SR"], meta["S"], meta["TPC"]
    NBLK, BLKR = meta["NBLK"], meta["BLKR"]
    GP, NGT = meta["GP"], meta["NGT"]
    TOTCH, TOT16, CHMAX = meta["TOTCH"], meta["TOT16"], meta["CHMAX"]
    DIN = meta["DIN"]
    H = weights["W1"].shape[1]
    OUT = weights["Wl"].shape[1]
    n_graphs = meta["n_graphs"]
    has_b = [bool(np.any(weights[k])) for k in ("b1", "b2", "b3")]
    has_bl = bool(np.any(weights["bl"]))

    DT = mybir.dt.bfloat16 if use_bf16 else F32
    nc = bacc.Bacc("TRN2", target_bir_lowering=False, debug=False,
                   num_devices=NCORES, num_swdge_queues=4)

    # ---- I/O tensors
    t_xT = nc.dram_tensor("xT", [DIN, S], F32, kind="ExternalInput")
    t_idx = nc.dram_tensor("idx", [P, TOT16], I16, kind="ExternalInput")
    t_dslot = nc.dram_tensor("dslot", [P, TOTCH], F32, kind="ExternalInput")
    t_deg = nc.dram_tensor("deg", [P, TPC], F32, kind="ExternalInput")
    t_pslot = nc.dram_tensor("pslot", [P, TPC], F32, kind="ExternalInput")
    t_cnt = nc.dram_tensor("cnt", [P, NGT], F32, kind="ExternalInput")
    t_iota = nc.dram_tensor("iota", [P, GP], F32, kind="ExternalInput")
    t_W = {}
    for wn, shp in (("W1", [DIN, H]), ("W2", [H, H]), ("W3", [H, H]),
                    ("Wl", [H, OUT])):
        t_W[wn] = nc.dram_tensor(wn, shp, F32, kind="ExternalInput")
    t_b = {}
    for bn in ("b1", "b2", "b3"):
        t_b[bn] = nc.dram_tensor(bn, [P, H], F32, kind="ExternalInput")
    t_bl = nc.dram_tensor("bl", [P, OUT], F32, kind="ExternalInput")
    t_out = nc.dram_tensor("out", [n_graphs, OUT], F32, kind="ExternalOutput")

    AOP = mybir.AluOpType
    ACT = mybir.ActivationFunctionType

    with tile.TileContext(nc, num_cores=NCORES) as tc:
        with tc.tile_pool(name="const", bufs=1) as cp, \
             tc.tile_pool(name="dram", bufs=1, space="DRAM") as dp:
            # ---- constants to SBUF
            iota_sb = cp.tile([P, GP], F32)
            nc.sync.dma_start(iota_sb[:], t_iota[:])
            idx_sb = cp.tile([P, TOT16], I16)
            nc.sync.dma_start(idx_sb[:], t_idx[:])
            dslot_sb = cp.tile([P, TOTCH], F32)
            nc.sync.dma_start(dslot_sb[:], t_dslot[:])
            deg_sb = cp.tile([P, TPC], F32)
            nc.sync.dma_start(deg_sb[:], t_deg[:])
            pslot_sb = cp.tile([P, TPC], F32)
            nc.sync.dma_start(pslot_sb[:], t_pslot[:])
            cnt_sb = cp.tile([P, NGT], F32)
            nc.sync.dma_start(cnt_sb[:], t_cnt[:])
            W_sb = {}
            for wn, t_w in t_W.items():
                W_sb[wn] = cp.tile(list(t_w.shape), F32, name=f"W_{wn}_sb")
                nc.sync.dma_start(W_sb[wn][:], t_w[:])
            b_sb = {}
            for i, bn in enumerate(("b1", "b2", "b3")):
                if has_b[i]:
                    b_sb[bn] = cp.tile([P, H], F32, name=f"b_{bn}_sb")
                    nc.sync.dma_start(b_sb[bn][:], t_b[bn][:])
            if has_bl:
                bl_sb = cp.tile([P, OUT], F32)
                nc.sync.dma_start(bl_sb[:], t_bl[:])
            ident_sb = cp.tile([P, P], F32)
            from concourse.masks import make_identity
            make_identity(nc, ident_sb[:])

            # dinv = 1/sqrt(deg)  (deg >= 1 always: self loops)
            dinv_sb = cp.tile([P, TPC], F32)
            nc.scalar.sqrt(dinv_sb[:], deg_sb[:])
            nc.vector.reciprocal(dinv_sb[:], dinv_sb[:])
            # cntinv = 1/max(cnt, 1)
            cntinv_sb = cp.tile([P, NGT], F32)
            nc.vector.tensor_scalar_max(cntinv_sb[:], cnt_sb[:], 1.0)
            nc.vector.reciprocal(cntinv_sb[:], cntinv_sb[:])

            # ---- DRAM scratch
            u_shard = dp.tile([S, H], DT)
            u_table = dp.tile([S * NCORES, H], DT)
            pool_dram = dp.tile([GP, H], F32)
            pool_ar = dp.tile([GP, H], F32)

            # ---- layer-1 u: u1 = dinv * (x @ W1), from xT shipped by host
            with tc.tile_pool(name="xTp", bufs=1) as xp, \
                 tc.tile_pool(name="u1w", bufs=4) as u1p, \
                 tc.tile_pool(name="u1ps", bufs=4, space="PSUM") as u1ps:
                xT_sb = xp.tile([DIN, S], F32)
                nc.sync.dma_start(xT_sb[:], t_xT[:])
                for t in range(TPC):
                    ps = u1ps.tile([P, H], F32, tag="ps")
                    nc.tensor.matmul(ps[:], lhsT=xT_sb[:, t * P:(t + 1) * P],
                                     rhs=W_sb["W1"][:], start=True, stop=True)
                    u_sb = u1p.tile([P, H], DT, tag="u")
                    nc.scalar.activation(u_sb[:], ps[:], ACT.Copy,
                                         scale=dinv_sb[:, t:t + 1])
                    nc.sync.dma_start(u_shard[t * P:(t + 1) * P, :], u_sb[:])

            # ---- main layer loop
            with tc.tile_pool(name="msg", bufs=msg_bufs) as mp, \
                 tc.tile_pool(name="stair", bufs=2) as sp, \
                 tc.tile_pool(name="work", bufs=3) as wp, \
                 tc.tile_pool(name="hps", bufs=2, space="PSUM") as hps, \
                 tc.tile_pool(name="tps", bufs=2, space="PSUM") as tps, \
                 tc.tile_pool(name="ups", bufs=2, space="PSUM") as ups, \
                 tc.tile_pool(name="pps", bufs=1, space="PSUM") as pps:
                pool_ps = pps.tile([P, GP], F32)
                n_layers = 3 if stage >= 3 else (1 if stage >= 1.5 else 0)
                for li in range(n_layers):
                    # AllGather this layer's u
                    nc.gpsimd.collective_compute(
                        "AllGather", AOP.bypass,
                        replica_groups=[list(range(NCORES))],
                        ins=[u_shard.opt()], outs=[u_table.opt()],
                    )
                    W_next = ("W2", "W3", None)[li]
                    for g in meta["groups"]:
                        nch = g["n_chunks"]
                        msg = mp.tile([P, CHMAX, H], DT, tag="msg")
                        for b_i, (off16, n_idx, local0) in enumerate(g["blocks"]):
                            if n_idx == 0 or stage < 2:
                                continue
                            nbch = n_idx // P
                            blk_hi = min((b_i + 1) * BLKR, S * NCORES)
                            nc.gpsimd.dma_gather(
                                out_ap=msg[:, local0:local0 + nbch, :],
                                in_ap=u_table[b_i * BLKR:blk_hi, :],
                                idxs_ap=idx_sb[:, off16:off16 + n_idx // 16],
                                num_idxs=n_idx,
                                num_idxs_reg=n_idx,
                                elem_size=H,
                                single_packet=False,
                                queue_num=b_i % 4,
                            )
                        for (t, tch, tmb) in g["tiles"]:
                            if not tch or stage < 3:
                                continue
                            ntc = len(tch)
                            # one-hot staircases for all chunks of this tile
                            stair = sp.tile([P, ntc, P], DT, tag="st")
                            nc.vector.tensor_tensor(
                                out=stair[:],
                                in0=iota_sb[:, 0:P].unsqueeze(1)
                                    .to_broadcast([P, ntc, P]),
                                in1=dslot_sb[:, tmb:tmb + ntc].unsqueeze(2)
                                    .to_broadcast([P, ntc, P]),
                                op=AOP.is_equal,
                            )
                            ps_h = hps.tile([P, H], F32, tag="h")
                            for k, lc in enumerate(tch):
                                nc.tensor.matmul(
                                    ps_h[:], lhsT=stair[:, k, :],
                                    rhs=msg[:, lc, :],
                                    start=(k == 0), stop=(k == ntc - 1))
                            # self-loop: add this tile's own u (local read)
                            u_loc = wp.tile([P, H], DT, tag="uloc")
                            nc.sync.dma_start(u_loc[:],
                                              u_shard[t * P:(t + 1) * P, :])
                            tmp = wp.tile([P, H], F32, tag="tmp")
                            nc.vector.tensor_tensor(
                                out=tmp[:], in0=ps_h[:], in1=u_loc[:],
                                op=AOP.add)
                            h_sb = wp.tile([P, H], F32, tag="h")
                            if has_b[li]:
                                nc.vector.tensor_scalar_mul(
                                    tmp[:], tmp[:], dinv_sb[:, t:t + 1])
                                nc.vector.tensor_tensor(
                                    out=tmp[:], in0=tmp[:],
                                    in1=b_sb[("b1", "b2", "b3")[li]][:],
                                    op=AOP.add)
                                nc.scalar.activation(h_sb[:], tmp[:], ACT.Relu)
                            else:
                                nc.scalar.activation(
                                    h_sb[:], tmp[:], ACT.Relu,
                                    scale=dinv_sb[:, t:t + 1])
                            if W_next is not None:
                                ps_t = tps.tile([P, P], F32, tag="t")
                                nc.tensor.transpose(ps_t[:], h_sb[:],
                                                    ident_sb[:])
                                hT_sb = wp.tile([P, P], F32, tag="ht")
                                nc.vector.tensor_copy(hT_sb[:], ps_t[:])
                                ps_u = ups.tile([P, H], F32, tag="u")
                                nc.tensor.matmul(ps_u[:], lhsT=hT_sb[:],
                                                 rhs=W_sb[W_next][:],
                                                 start=True, stop=True)
                                u_sb = wp.tile([P, H], DT, tag="u")
                                nc.scalar.activation(
                                    u_sb[:], ps_u[:], ACT.Copy,
                                    scale=dinv_sb[:, t:t + 1])
                                nc.sync.dma_start(
                                    u_shard[t * P:(t + 1) * P, :], u_sb[:])
                            else:
                                # pool: one-hot over global graph slots
                                stp = sp.tile([P, GP], F32, tag="stp")
                                nc.vector.tensor_tensor(
                                    out=stp[:], in0=iota_sb[:],
                                    in1=pslot_sb[:, t:t + 1]
                                        .to_broadcast([P, GP]),
                                    op=AOP.is_equal)
                                # start=True clears has_written for the WHOLE
                                # 2KB psum bank -> only the first matmul per
                                # bank may set it (4 halves of 128 f32 / bank).
                                for hh in range(NGT):
                                    nc.tensor.matmul(
                                        pool_ps[:, hh * P:(hh + 1) * P],
                                        lhsT=stp[:, hh * P:(hh + 1) * P],
                                        rhs=h_sb[:],
                                        start=(t == 0 and hh % 4 == 0),
                                        stop=(t == TPC - 1),
                                        skip_group_check=True)

                if stage < 4:
                    z_sb = wp.tile([P, OUT], F32, tag="o")
                    nc.vector.memset(z_sb[:], 0.0)
                    nc.sync.dma_start(t_out[0:min(P, n_graphs), :],
                                      z_sb[:min(P, n_graphs), :])
                else:
                    # ---- pool wrap-up: PSUM -> SBUF -> DRAM -> AllReduce
                    poolacc = wp.tile([P, GP], F32, tag="pa")
                    nc.vector.tensor_copy(poolacc[:], pool_ps[:])
                    nc.sync.dma_start(
                        pool_dram[:].rearrange("(h p) f -> p h f", p=P),
                        poolacc[:].rearrange("p (h f) -> p h f", h=NGT))
                    nc.gpsimd.collective_compute(
                        "AllReduce", AOP.add,
                        replica_groups=[list(range(NCORES))],
                        ins=[pool_dram.opt()], outs=[pool_ar.opt()],
                    )
                    # ---- final linear on pooled means (replicated)
                    for gt in range(NGT):
                        pt = wp.tile([P, H], F32, tag="pt")
                        nc.sync.dma_start(pt[:],
                                          pool_ar[gt * P:(gt + 1) * P, :])
                        nc.vector.tensor_scalar_mul(pt[:], pt[:],
                                                    cntinv_sb[:, gt:gt + 1])
                        ps_t = tps.tile([P, P], F32, tag="t")
                        nc.tensor.transpose(ps_t[:], pt[:], ident_sb[:])
                        ptT = wp.tile([P, P], F32, tag="ptT")
                        nc.vector.tensor_copy(ptT[:], ps_t[:])
                        ps_o = ups.tile([P, OUT], F32, tag="u")
                        nc.tensor.matmul(ps_o[:], lhsT=ptT[:],
                                         rhs=W_sb["Wl"][:],
                                         start=True, stop=True)
                        o_sb = wp.tile([P, OUT], F32, tag="o")
                        if has_bl:
                            nc.vector.tensor_tensor(out=o_sb[:], in0=ps_o[:],
                                                    in1=bl_sb[:], op=AOP.add)
                        else:
                            nc.vector.tensor_copy(o_sb[:], ps_o[:])
                        rows = min(P, n_graphs - gt * P)
                        nc.sync.dma_start(t_out[gt * P:gt * P + rows, :],
                                          o_sb[:rows, :])

    nc.finalize()
    return nc


# ---------------------------------------------------------------------------
# v2: hT-orientation, quarter-wise AllGather, 4 SWDGE queues, diag self-loops.
#
# Math (b1=b2=b3=0): per GCN layer out = dinv * (E + I) * u where
# u = dinv * (h @ W) and E is the plain edge-sum of u[src].  relu commutes
# with the positive per-node dinv, so h_next = relu(dinv*agg) and
# u_next = dinv * (h_next @ W) = dinv^2 * (relu(agg) @ W): all per-node
# scales live in per-partition ACT scales at u-write time; the staircase and
# self-loop diag stay exact 0/1 masks.
# ---------------------------------------------------------------------------

F16 = mybir.dt.float16


def _quarters(TPC):
    qtiles = [25, 25, 24, 24]
    assert sum(qtiles) == TPC
    qstart_t, acc = [], 0
    for qt in qtiles:
        qstart_t.append(acc)
        acc += qt
    qstart = [q * P for q in qstart_t]           # row starts in shard
    qrows = [qt * P for qt in qtiles]            # rows per quarter
    B, acc = [], 0
    for qr in qrows:
        B.append(acc)
        acc += NCORES * qr                       # table block bases
    return qtiles, qstart_t, qstart, qrows, B


def _preprocess2(x, edge_index, batch, n_graphs):
    N = x.shape[0]
    DIN = x.shape[1]
    SR = N // NCORES
    TPC = _ceil_div(SR, P)
    S = TPC * P
    GP = _ceil_div(n_graphs, P) * P
    NGT = GP // P
    qtiles, qstart_t, qstart, qrows, B = _quarters(TPC)

    src = np.asarray(edge_index[0], dtype=np.int64)
    dst = np.asarray(edge_index[1], dtype=np.int64)
    batch = np.asarray(batch, dtype=np.int64)
    deg = (np.bincount(dst, minlength=N) + 1).astype(np.float32)
    dinv_all = 1.0 / np.sqrt(deg)

    c_e = dst // SR
    drow = dst % SR
    t_e = drow // P
    v_e = (drow % P).astype(np.float32)
    r_e = src % SR
    sc_e = src // SR
    q_e = np.digitize(r_e, qstart[1:])
    qr = np.asarray(qrows)
    qs = np.asarray(qstart)
    idx_e = (sc_e * qr[q_e] + (r_e - qs[q_e])).astype(np.int64)
    assert idx_e.max() < 32768

    counts = np.zeros((NCORES, TPC, 4), dtype=np.int64)
    np.add.at(counts, (c_e, t_e, q_e), 1)
    ch = np.ceil(counts.max(axis=0) / P).astype(np.int64)      # [TPC, 4]
    ntc = 1 + ch.sum(axis=1)                                    # [TPC]
    tmbase = np.concatenate([[0], np.cumsum(ntc)[:-1]])
    TOTCH = int(ntc.sum())
    TOTI = int(ch.sum()) * P
    TOT16 = TOTI // 16

    # quarter-aligned groups of up to 4 tiles
    gtiles = []
    for qi in range(4):
        a, b = qstart_t[qi], qstart_t[qi] + qtiles[qi]
        for s0 in range(a, b, 4):
            gtiles.append(list(range(s0, min(s0 + 4, b))))
    groups = []
    idx_cursor16 = 0
    CHMAX = 0
    for tiles in gtiles:
        T_g = len(tiles)
        local = T_g
        qmeta = []           # per q: (off16, n_idx, local0)
        slot_of = {}         # (t, q, k) -> msg slot
        for q in range(4):
            n_idx = int(ch[[t for t in tiles], q].sum()) * P
            qmeta.append((idx_cursor16, n_idx, local))
            for t in tiles:
                for k in range(int(ch[t, q])):
                    slot_of[(t, q, k)] = local
                    local += 1
            idx_cursor16 += n_idx // 16
        # split-balance the 4 quarter streams across the 4 SWDGE queues:
        # each call is (src_quarter, off16, n_idx, local0, queue).
        loads = [n for (_, n, _) in qmeta]
        mean = sum(loads) // 4
        calls = []
        spare = []           # (q, off16, n_idx, local0) pieces for light qs
        for q in range(4):
            off16, n_idx, local0 = qmeta[q]
            if n_idx > mean + P and n_idx - mean >= 2 * P:
                keep = max(P, (mean // P) * P)
                rest = n_idx - keep
                calls.append((q, off16, keep, local0, q))
                spare.append((q, off16 + keep // 16, rest,
                              local0 + keep // P))
            elif n_idx > 0:
                calls.append((q, off16, n_idx, local0, q))
        qload = [0, 0, 0, 0]
        for (_, _, n, _, qq) in calls:
            qload[qq] += n
        for piece in sorted(spare, key=lambda x: -x[2]):
            qq = qload.index(min(qload))
            calls.append((piece[0], piece[1], piece[2], piece[3], qq))
            qload[qq] += piece[2]
        tiles_meta = []
        for j, t in enumerate(tiles):
            slots = [j]
            for q in range(4):
                for k in range(int(ch[t, q])):
                    slots.append(slot_of[(t, q, k)])
            tiles_meta.append((t, slots, int(tmbase[t])))
        groups.append({"tiles": tiles, "tiles_meta": tiles_meta,
                       "calls": calls, "qmeta": qmeta, "n_chunks": local})
        CHMAX = max(CHMAX, local)
    assert idx_cursor16 == TOT16

    import ml_dtypes
    BF = ml_dtypes.bfloat16
    cnt = np.bincount(batch, minlength=n_graphs).astype(np.float32)
    cntinv_flat = np.ones(GP, dtype=np.float32)
    cntinv_flat[:n_graphs] = 1.0 / np.maximum(cnt, 1.0)
    cntinv_w = cntinv_flat.reshape(NGT, P).T.copy()
    iota_p = np.broadcast_to(np.arange(P, dtype=np.float32)[None, :],
                             (P, P)).astype(BF).copy()

    in_maps = []
    for c in range(NCORES):
        m = c_e == c
        et, eq, ei, ev = t_e[m], q_e[m], idx_e[m], v_e[m]
        order = np.lexsort((eq, et))
        et, eq, ei, ev = et[order], eq[order], ei[order], ev[order]
        key = et * 4 + eq
        bnd = np.searchsorted(key, np.arange(TPC * 4 + 1))

        idx_vals = np.zeros(TOTI, dtype=np.int16)
        dslot_vals = np.full((TOTCH, P), -1.0, dtype=np.float32)
        dslot_vals[tmbase, :] = np.arange(P, dtype=np.float32)[None, :]
        dpos = np.ones(TPC, dtype=np.int64)   # col cursor per tile (0=self)
        pos = 0
        for g in groups:
            for q in range(4):
                off16, n_idx, local0 = g["qmeta"][q]
                assert pos == off16 * 16
                for t in g["tiles"]:
                    lo, hi = bnd[t * 4 + q], bnd[t * 4 + q + 1]
                    n_pad = int(ch[t, q]) * P
                    if n_pad == 0:
                        assert hi == lo
                        continue
                    seg = np.zeros(n_pad, dtype=np.int16)
                    seg[:hi - lo] = ei[lo:hi].astype(np.int16)
                    idx_vals[pos:pos + n_pad] = seg
                    dcol = tmbase[t] + dpos[t]
                    nchk = int(ch[t, q])
                    dv = np.full(n_pad, -1.0, dtype=np.float32)
                    dv[:hi - lo] = ev[lo:hi]
                    dslot_vals[dcol:dcol + nchk, :] = dv.reshape(nchk, P)
                    dpos[t] += nchk
                    pos += n_pad
        assert pos == TOTI
        idx_w = np.tile(idx_vals.reshape(-1, 16).T.copy(), (8, 1))

        nodes = np.arange(c * SR, (c + 1) * SR)
        dinv_flat = np.ones(S, dtype=np.float32)
        dinv_flat[:SR] = dinv_all[nodes]
        dinv_w = dinv_flat.reshape(TPC, P).T.copy()
        # pool staircase shipped precomputed: dinv-weighted one-hot over
        # global graph slots, [P, TPC*GP] bf16 (tile-major columns).
        pz = np.zeros((S, GP), dtype=np.float32)
        pz[np.arange(SR), batch[nodes]] = dinv_flat[:SR]
        pstair_w = pz.reshape(TPC, P, GP).transpose(1, 0, 2).reshape(
            P, TPC * GP).astype(BF)
        # sharded x, pre-scaled by dinv (u1 = xs @ W1 needs no on-dev scale)
        xT = np.zeros((DIN, S), dtype=np.float32)
        xT[:, :SR] = (np.asarray(x[nodes], dtype=np.float32)
                      * dinv_all[nodes][:, None]).T

        # edge staircases shipped precomputed (identical all 3 sweeps):
        # [P, TOTCH*P] bf16, dslot-column (tile-) major.
        estair_w = (dslot_vals[:, :, None]
                    == np.arange(P, dtype=np.float32)[None, None, :]
                    ).transpose(1, 0, 2).reshape(P, TOTCH * P).astype(BF)

        in_maps.append({
            "xT": xT.astype(BF), "idx": idx_w,
            "estair": estair_w,
            "dinv2": (dinv_w * dinv_w).copy(),
            "pstair": pstair_w, "cntinv": cntinv_w,
        })

    # full x in u-table row order (quarter-block major), dinv-prescaled:
    # every core computes the whole u1 table locally (no prestage AGs).
    xs_all = np.asarray(x, dtype=np.float32) * dinv_all[:, None]
    xTf = np.zeros((DIN, NCORES * S), dtype=np.float32)
    pos = 0
    for q in range(4):
        for c in range(NCORES):
            lo = c * SR + qstart[q]
            nreal = min(SR, qstart[q] + qrows[q]) - qstart[q]
            xTf[:, pos:pos + nreal] = xs_all[lo:lo + nreal].T
            pos += qrows[q]
    assert pos == NCORES * S
    xTf = xTf.astype(BF)
    for m in in_maps:
        m["xTf"] = xTf

    meta = dict(N=N, DIN=DIN, SR=SR, S=S, TPC=TPC, GP=GP, NGT=NGT,
                qstart=qstart, qrows=qrows, B=B, qstart_t=qstart_t,
                qtiles=qtiles, groups=groups, TOTCH=TOTCH, TOT16=TOT16,
                CHMAX=CHMAX, n_graphs=n_graphs)
    return in_maps, meta


def _build2(meta, weights):
    S, TPC, GP, NGT = meta["S"], meta["TPC"], meta["GP"], meta["NGT"]
    TOTCH, TOT16, CHMAX = meta["TOTCH"], meta["TOT16"], meta["CHMAX"]
    DIN = meta["DIN"]
    qstart, qrows, B = meta["qstart"], meta["qrows"], meta["B"]
    qstart_t, qtiles = meta["qstart_t"], meta["qtiles"]
    H = weights["W2"].shape[0]
    OUT = weights["Wl"].shape[1]
    n_graphs = meta["n_graphs"]
    BF16 = mybir.dt.bfloat16
    AOP = mybir.AluOpType
    ACT = mybir.ActivationFunctionType
    qend_t = [qstart_t[q] + qtiles[q] - 1 for q in range(4)]   # last tile/q

    nc = bacc.Bacc("TRN2", target_bir_lowering=False, debug=False,
                   num_devices=NCORES, num_swdge_queues=4)

    t_xT = nc.dram_tensor("xT", [DIN, S], BF16, kind="ExternalInput")
    t_xTf = nc.dram_tensor("xTf", [DIN, NCORES * S], BF16,
                           kind="ExternalInput")
    t_idx = nc.dram_tensor("idx", [P, TOT16], I16, kind="ExternalInput")
    t_estair = nc.dram_tensor("estair", [P, TOTCH * P], BF16,
                              kind="ExternalInput")
    t_dinv2 = nc.dram_tensor("dinv2", [P, TPC], F32, kind="ExternalInput")
    t_pstair = nc.dram_tensor("pstair", [P, TPC * GP], BF16,
                              kind="ExternalInput")
    t_cntinv = nc.dram_tensor("cntinv", [P, NGT], F32, kind="ExternalInput")
    t_W = {}
    for wn, shp in (("W1", [DIN, H]), ("W2", [H, H]), ("W3", [H, H]),
                    ("Wl", [H, OUT])):
        t_W[wn] = nc.dram_tensor(wn, shp, BF16, kind="ExternalInput")
    t_bl = nc.dram_tensor("bl", [P, OUT], F32, kind="ExternalInput")
    t_out = nc.dram_tensor("out", [n_graphs, OUT], F32, kind="ExternalOutput")

    # AG slices (tile ranges).  Quarters 0-2 AllGather whole (contiguous
    # output); quarter 3 is sliced so the sweep boundary only exposes the
    # last tiny slice -- sliced AGs land in contiguous staging tensors
    # and are relaid into the quarter block by a strided DMA.
    import os as _os
    if _os.environ.get("AG_QUARTERS", "0") == "1":
        ag_slices = [(0, 25), (25, 50), (50, 74), (74, 98)]
    else:
        ag_slices = [(0, 25), (25, 50), (50, 74), (74, 86), (86, 94),
                     (94, 98)]

    with tile.TileContext(nc, num_cores=NCORES) as tc:
        with tc.tile_pool(name="const", bufs=1) as cp, \
             tc.tile_pool(name="bncp", bufs=1) as bp_sb, \
             tc.tile_pool(name="dram", bufs=1, space="DRAM") as dp, \
             tc.tile_pool(name="dshr", bufs=1, space="DRAM") as dsp:
            # prestage-critical loads first; bulky gather tables after.
            W_sb = {}
            for wn, t_w in t_W.items():
                W_sb[wn] = cp.tile(list(t_w.shape), BF16, name=f"W_{wn}_sb")
                nc.sync.dma_start(W_sb[wn][:], t_w[:])
            idx_sb = cp.tile([P, TOT16], I16)
            nc.sync.dma_start(idx_sb[:], t_idx[:])
            dinv2_sb = cp.tile([P, TPC], F32)
            nc.sync.dma_start(dinv2_sb[:], t_dinv2[:])
            cntinv_sb = cp.tile([P, NGT], F32)
            nc.sync.dma_start(cntinv_sb[:], t_cntinv[:])
            bl_sb = cp.tile([P, OUT], F32)
            nc.sync.dma_start(bl_sb[:], t_bl[:])

            u_shard = [dp.tile([S, H], BF16, name=f"u_shard{i}")
                       for i in range(2)]
            u_table = [[dp.tile([NCORES * qrows[q], H], BF16,
                                name=f"u_table{i}_{q}")
                        for q in range(4)] for i in range(3)]
            ag_stage = [[dp.tile(
                            [NCORES * (ag_slices[si][1] - ag_slices[si][0])
                             * P, H], BF16, name=f"ag_stage{i}_{si}")
                         for si in range(3, len(ag_slices))]
                        for i in range(3)]
            pool_dram = dp.tile([P, GP], BF16)
            pool_ar = dsp.tile([P, GP], BF16, addr_space="Shared")

            def emit_ag_slice(si, stage_i, shard, tables):
                ta, tb = ag_slices[si]
                q = next(qi for qi in range(4)
                         if qstart_t[qi] <= ta < qstart_t[qi] + qtiles[qi])
                assert tb <= qstart_t[q] + qtiles[q]
                r0 = ta * P - qstart[q]          # row offset within quarter
                nr = (tb - ta) * P
                if r0 == 0 and nr == qrows[q]:
                    out_ap = tables[q][:].opt()
                    nc.gpsimd.collective_compute(
                        "AllGather", AOP.bypass,
                        replica_groups=[list(range(NCORES))],
                        ins=[shard[ta * P:tb * P, :].opt()],
                        outs=[out_ap],
                    )
                else:
                    st = ag_stage[stage_i][si - 3]
                    nc.gpsimd.collective_compute(
                        "AllGather", AOP.bypass,
                        replica_groups=[list(range(NCORES))],
                        ins=[shard[ta * P:tb * P, :].opt()],
                        outs=[st[:].opt()],
                    )
                    # bounce through SBUF (collective outs must be
                    # contiguous; DRAM->DRAM relayout is not safe).  On
                    # the scalar HWDGE queue to keep Sync free.
                    nt = nr // P
                    bt = bp_sb.tile([P, NCORES * nt, H], BF16, tag="bnc",
                                    name=f"bnc_{stage_i}_{si}")
                    nc.scalar.dma_start(
                        bt[:], st[:].rearrange("(b p) f -> p b f", p=P))
                    for c in range(NCORES):
                        nc.scalar.dma_start(
                            tables[q][:]
                            .rearrange("(c r) f -> c r f", c=NCORES)
                            [c, r0:r0 + nr, :]
                            .rearrange("(b p) f -> p b f", p=P),
                            bt[:, c * nt:(c + 1) * nt, :])

            # ---- layer-1 prestage: u1 = xs @ W1 (x pre-scaled by dinv on
            # host).  Every core computes the FULL u1 table locally from
            # replicated xTf -- no collectives; plus its own shard from xT.
            with tc.tile_pool(name="xp", bufs=1) as xp, \
                 tc.tile_pool(name="xw", bufs=6) as xw, \
                 tc.tile_pool(name="u1w", bufs=4) as u1w, \
                 tc.tile_pool(name="u1ps", bufs=4, space="PSUM") as u1ps:
                xT_sb = xp.tile([DIN, S], BF16)
                nc.sync.dma_start(xT_sb[:], t_xT[:])
                # table block boundaries are 512-aligned: windows of 4 tiles
                WT = NCORES * S // 512
                for w in range(WT):
                    xf = xw.tile([DIN, 512], BF16, tag="x")
                    nc.sync.dma_start(xf[:],
                                      t_xTf[:, w * 512:(w + 1) * 512])
                    ps = u1ps.tile([P, 4, H], F32, tag="ps")
                    for j in range(4):
                        nc.tensor.matmul(
                            ps[:, j, :], lhsT=xf[:, j * P:(j + 1) * P],
                            rhs=W_sb["W1"][:], start=True, stop=True)
                    u_sb = u1w.tile([P, 4, H], BF16, tag="u")
                    nc.vector.tensor_copy(u_sb[:], ps[:])
                    q = next(qi for qi in range(4)
                             if B[qi] <= w * 512 < B[qi] + NCORES * qrows[qi])
                    nc.sync.dma_start(
                        u_table[0][q][w * 512 - B[q]:
                                      (w + 1) * 512 - B[q], :]
                        .rearrange("(j p) f -> p j f", p=P),
                        u_sb[:])
                # own shard (for sweep-0 self-loop chunks)
                for w in range(TPC // 2):
                    ps = u1ps.tile([P, 2, H], F32, tag="ps2")
                    for j in range(2):
                        t = w * 2 + j
                        nc.tensor.matmul(
                            ps[:, j, :], lhsT=xT_sb[:, t * P:(t + 1) * P],
                            rhs=W_sb["W1"][:], start=True, stop=True)
                    u_sb = u1w.tile([P, 2, H], BF16, tag="u2")
                    nc.vector.tensor_copy(u_sb[:], ps[:])
                    nc.sync.dma_start(
                        u_shard[0][w * 2 * P:(w + 1) * 2 * P, :]
                        .rearrange("(j p) f -> p j f", p=P),
                        u_sb[:])

            # ---- main sweeps
            with tc.tile_pool(name="msg", bufs=3) as mp, \
                 tc.tile_pool(name="stair", bufs=2) as sp, \
                 tc.tile_pool(name="work", bufs=4) as wp, \
                 tc.tile_pool(name="hsb", bufs=10) as hp, \
                 tc.tile_pool(name="pstp", bufs=6) as pp_sb, \
                 tc.tile_pool(name="hps", bufs=4, space="PSUM") as hps, \
                 tc.tile_pool(name="ups", bufs=2, space="PSUM") as ups, \
                 tc.tile_pool(name="pps", bufs=1, space="PSUM") as pps:
                pool_ps = pps.tile([P, GP], F32)
                # slice si (last tile in group j) is emitted after group
                # j+4 (msg-pool depth + 1 deferred group) so its u-writes
                # are done when the Pool engine reaches the AG.
                ngrp = len(meta["groups"])
                emit_after = [[] for _ in range(ngrp)]
                for si, (ta, tb) in enumerate(ag_slices):
                    j = next(gi for gi, g in enumerate(meta["groups"])
                             if (tb - 1) in g["tiles"])
                    emit_after[min(j + 4, ngrp - 1)].append(si)

                def flush(s, pend, nxt, W_next):
                    """Deferred per-tile PE stage (one group behind)."""
                    for t, tsb, pst in pend:
                        if s < 2:
                            ps_u = ups.tile([P, H], F32, tag="u")
                            nc.tensor.matmul(ps_u[:], lhsT=tsb[:],
                                             rhs=W_sb[W_next][:],
                                             start=True, stop=True)
                            u_sb = wp.tile([P, H], BF16, tag="us")
                            nc.scalar.activation(
                                u_sb[:], ps_u[:], ACT.Copy,
                                scale=dinv2_sb[:, t:t + 1])
                            nc.sync.dma_start(
                                u_shard[nxt][t * P:(t + 1) * P, :],
                                u_sb[:])
                        else:
                            for hb in range(2):
                                nc.tensor.matmul(
                                    pool_ps[:, hb * 512:(hb + 1) * 512],
                                    lhsT=tsb[:],
                                    rhs=pst[:, hb * 512:(hb + 1) * 512],
                                    start=(t == 0), stop=(t == TPC - 1),
                                    skip_group_check=True)

                for s in range(3):
                    cur, nxt = (0, 1, 0)[s], (1, 0, None)[s]
                    W_next = ("W2", "W3", None)[s]
                    pend = []
                    for gi, g in enumerate(meta["groups"]):
                        tiles = g["tiles"]
                        T_g = len(tiles)
                        t0 = tiles[0]
                        msg = mp.tile([P, CHMAX, H], BF16, tag="msg")
                        for (q, off16, n_idx, local0, qq) in g["calls"]:
                            nc.gpsimd.dma_gather(
                                out_ap=msg[:, local0:local0 + n_idx // P, :],
                                in_ap=u_table[s][q][:],
                                idxs_ap=idx_sb[:, off16:off16 + n_idx // 16],
                                num_idxs=n_idx,
                                num_idxs_reg=n_idx,
                                elem_size=H,
                                single_packet=False,
                                queue_num=qq,
                            )
                        nc.sync.dma_start(
                            msg[:, 0:T_g, :],
                            u_shard[cur][t0 * P:(t0 + T_g) * P, :]
                            .rearrange("(j p) f -> p j f", p=P))
                        pend_next = []
                        tmb0 = g["tiles_meta"][0][2]
                        nchg = g["n_chunks"]
                        est = sp.tile([P, CHMAX, P], BF16, tag="est")
                        nc.sync.dma_start(
                            est[:, 0:nchg, :],
                            t_estair[:, tmb0 * P:(tmb0 + nchg) * P]
                            .rearrange("p (c v) -> p c v", v=P))
                        for (t, slots, tmb) in g["tiles_meta"]:
                            ntc = len(slots)
                            eb = tmb - tmb0
                            ps = hps.tile([P, H], F32, tag="h")
                            for k, lc in enumerate(slots):
                                if s < 2:
                                    nc.tensor.matmul(
                                        ps[:], lhsT=msg[:, lc, :],
                                        rhs=est[:, eb + k, :],
                                        start=(k == 0), stop=(k == ntc - 1))
                                else:
                                    nc.tensor.matmul(
                                        ps[:], lhsT=est[:, eb + k, :],
                                        rhs=msg[:, lc, :],
                                        start=(k == 0), stop=(k == ntc - 1))
                            tsb = hp.tile([P, P], BF16, tag="t",
                                          name=f"tsb_{s}_{t}")
                            nc.scalar.activation(tsb[:], ps[:], ACT.Relu)
                            pst = None
                            if s == 2:
                                pst = pp_sb.tile([P, GP], BF16, tag="pst",
                                                 name=f"pst_{t}")
                                nc.sync.dma_start(
                                    pst[:],
                                    t_pstair[:, t * GP:(t + 1) * GP])
                            pend_next.append((t, tsb, pst))
                        flush(s, pend, nxt, W_next)
                        pend = pend_next
                        # last group's AGs must follow the final flush
                        # (deferred u-writes) below, not precede it.
                        if s < 2 and gi < ngrp - 1:
                            for si in emit_after[gi]:
                                emit_ag_slice(si, s + 1, u_shard[nxt],
                                              u_table[s + 1])
                    flush(s, pend, nxt, W_next)
                    if s < 2:
                        for si in emit_after[ngrp - 1]:
                            emit_ag_slice(si, s + 1, u_shard[nxt],
                                          u_table[s + 1])

                # ---- pool wrap-up
                poolacc = wp.tile([P, GP], BF16, tag="pa")
                nc.vector.tensor_copy(poolacc[:], pool_ps[:])
                nc.sync.dma_start(pool_dram[:], poolacc[:])
                nc.gpsimd.collective_compute(
                    "AllReduce", AOP.add,
                    replica_groups=[list(range(NCORES))],
                    ins=[pool_dram.opt()], outs=[pool_ar.opt()],
                )
                pbf_sb = wp.tile([P, GP], BF16, tag="pbf")
                nc.sync.dma_start(pbf_sb[:], pool_ar[:])
                for hh in range(NGT):
                    ps_o = ups.tile([P, OUT], F32, tag="u")
                    nc.tensor.matmul(ps_o[:],
                                     lhsT=pbf_sb[:, hh * P:(hh + 1) * P],
                                     rhs=W_sb["Wl"][:], start=True, stop=True)
                    o_sb = wp.tile([P, OUT], F32, tag="o")
                    nc.scalar.activation(o_sb[:], ps_o[:], ACT.Copy,
                                         scale=cntinv_sb[:, hh:hh + 1])
                    o2_sb = wp.tile([P, OUT], F32, tag="o2")
                    nc.vector.tensor_tensor(out=o2_sb[:], in0=o_sb[:],
                                            in1=bl_sb[:], op=AOP.add)
                    rows = min(P, n_graphs - hh * P)
                    nc.sync.dma_start(t_out[hh * P:hh * P + rows, :],
                                      o2_sb[:rows, :])

    nc.finalize()
    return nc


def kernel(x, edge_index, batch, W1, b1, W2, b2, W3, b3, Wl, bl,
           group_tiles=4, trace=False, n_graphs=1000, stage=4, use_bf16=True):
    weights = dict(W1=np.asarray(W1, np.float32), b1=np.asarray(b1, np.float32),
                   W2=np.asarray(W2, np.float32), b2=np.asarray(b2, np.float32),
                   W3=np.asarray(W3, np.float32), b3=np.asarray(b3, np.float32),
                   Wl=np.asarray(Wl, np.float32), bl=np.asarray(bl, np.float32))
    if not (np.any(weights["b1"]) or np.any(weights["b2"])
            or np.any(weights["b3"])):
        return _kernel_v2(weights, x, edge_index, batch, n_graphs, trace)
    in_maps, meta = _preprocess(np.asarray(x, np.float32),
                                np.asarray(edge_index), np.asarray(batch),
                                n_graphs, group_tiles=group_tiles)
    nc = _build(meta, weights, stage=stage, use_bf16=use_bf16)
    # broadcast weight tensors (same on every core)
    H = weights["W1"].shape[1]
    OUT = weights["Wl"].shape[1]
    for m in in_maps:
        for wn in ("W1", "W2", "W3", "Wl"):
            m[wn] = weights[wn]
        for bn in ("b1", "b2", "b3"):
            m[bn] = np.broadcast_to(weights[bn][None, :], (P, H)).copy()
        m["bl"] = np.broadcast_to(weights["bl"][None, :], (P, OUT)).copy()
    res = run_bass_kernel_spmd(nc, in_maps, core_ids=list(range(NCORES)),
                               trace=trace)
    kernel.last_result = res
    return res.results[0]["out"][:n_graphs].astype(np.float32)


def _kernel_v2(weights, x, edge_index, batch, n_graphs, trace):
    import ml_dtypes
    BF = ml_dtypes.bfloat16
    in_maps, meta = _preprocess2(np.asarray(x, np.float32),
                                 np.asarray(edge_index), np.asarray(batch),
                                 n_graphs)
    nc = _build2(meta, weights)
    OUT = weights["Wl"].shape[1]
    for m in in_maps:
        for wn in ("W1", "W2", "W3", "Wl"):
            m[wn] = weights[wn].astype(BF)
        m["bl"] = np.broadcast_to(weights["bl"][None, :], (P, OUT)).astype(
            np.float32).copy()
    res = run_bass_kernel_spmd(nc, in_maps, core_ids=list(range(NCORES)),
                               trace=trace)
    kernel.last_result = res
    return res.results[0]["out"][:n_graphs].astype(np.float32)

